# revision 1
# baseline (speedup 1.0000x reference)
"""Distributed kernel for nn_AugmentedGeometryScaledDotProductAttention.

Strategy: pure data-parallel over batch. B=8 batch elements map 1:1 onto the
8 trn2 NeuronCores (jax axon devices). Each core runs the full per-batch
computation (geometry bias + 16-head attention + output projection) on its
own batch element; results are gathered back to a full (8, 512, 1024) output.
No cross-core collectives are needed: fc_o only needs the 16 heads of its own
batch element, which are all resident on the same core.

Self-contained: all shapes/constants hardcoded from the problem spec.
"""

import functools

import jax
import jax.numpy as jnp
import numpy as np

D_MODEL = 1024
H = 16
D_K = 64
D_V = 64
D_G = D_MODEL // H  # 64
WAVE_LEN = 1000.0
B = 8
N = 512
N_CORES = 8


def _box_relational_embedding(boxes):
    # boxes: (n, 4) for a single batch element
    x_min, y_min, x_max, y_max = jnp.split(boxes, 4, axis=-1)  # (n, 1)
    cx = (x_min + x_max) * 0.5
    cy = (y_min + y_max) * 0.5
    w = (x_max - x_min) + 1.0
    h = (y_max - y_min) + 1.0
    delta_x = jnp.log(jnp.clip(jnp.abs((cx - cx.T) / w), 1e-3, None))
    delta_y = jnp.log(jnp.clip(jnp.abs((cy - cy.T) / h), 1e-3, None))
    delta_w = jnp.log(w / w.T)
    delta_h = jnp.log(h / h.T)
    pos = jnp.stack([delta_x, delta_y, delta_w, delta_h], axis=-1)  # (n, n, 4)
    n_freq = D_G // 8
    feat_range = jnp.arange(n_freq, dtype=jnp.float32)
    dim_mat = 1.0 / (WAVE_LEN ** (feat_range / n_freq))
    mul = (100.0 * pos)[..., None] * dim_mat  # (n, n, 4, n_freq)
    n = pos.shape[0]
    mul = mul.reshape(n, n, 4 * n_freq)
    return jnp.concatenate([jnp.sin(mul), jnp.cos(mul)], axis=-1)  # (n, n, D_G)


def _per_batch(q_in, k_in, v_in, boxes, Wq, bq, Wk, bk, Wv, bv, Wo, bo, Wg, bg):
    # q_in/k_in/v_in: (N, D_MODEL); boxes: (N, 4) — one batch element.
    emb = _box_relational_embedding(boxes)  # (N, N, D_G)
    g = jax.nn.relu(jnp.einsum("nmd,hd->hnm", emb, Wg) + bg[:, None, None])
    q = (q_in @ Wq.T + bq).reshape(N, H, D_K).transpose(1, 0, 2)  # (H, N, D_K)
    k = (k_in @ Wk.T + bk).reshape(N, H, D_K).transpose(1, 0, 2)
    v = (v_in @ Wv.T + bv).reshape(N, H, D_V).transpose(1, 0, 2)
    a = jnp.einsum("hqd,hkd->hqk", q, k) / jnp.sqrt(jnp.float32(D_K))
    mn = jax.nn.softmax(jnp.log(jnp.clip(g, 1e-6, None)) + a, axis=-1)
    out = jnp.einsum("hqk,hkd->qhd", mn, v).reshape(N, H * D_V)
    return out @ Wo.T + bo  # (N, D_MODEL)


@functools.partial(
    jax.pmap,
    axis_name="cores",
    in_axes=(0, 0, 0, 0) + (None,) * 10,
    out_axes=0,
)
def _pmapped(queries, keys, values, boxes, Wq, bq, Wk, bk, Wv, bv, Wo, bo, Wg, bg):
    return _per_batch(
        queries, keys, values, boxes, Wq, bq, Wk, bk, Wv, bv, Wo, bo, Wg, bg
    )


def kernel(
    queries, keys, values, boxes, Wq, bq, Wk, bk, Wv, bv, Wo, bo, Wg, bg
) -> np.ndarray:
    """Full inputs in, full output out. Shards batch across the 8 NeuronCores."""
    out = _pmapped(
        jnp.asarray(queries, jnp.float32),
        jnp.asarray(keys, jnp.float32),
        jnp.asarray(values, jnp.float32),
        jnp.asarray(boxes, jnp.float32),
        jnp.asarray(Wq, jnp.float32),
        jnp.asarray(bq, jnp.float32),
        jnp.asarray(Wk, jnp.float32),
        jnp.asarray(bk, jnp.float32),
        jnp.asarray(Wv, jnp.float32),
        jnp.asarray(bv, jnp.float32),
        jnp.asarray(Wo, jnp.float32),
        jnp.asarray(bo, jnp.float32),
        jnp.asarray(Wg, jnp.float32),
        jnp.asarray(bg, jnp.float32),
    )
    return np.asarray(out, dtype=np.float32)  # (B, N, D_MODEL)


if __name__ == "__main__":
    rng = np.random.default_rng(0)
    demo = kernel(
        queries=rng.standard_normal((B, N, D_MODEL), dtype=np.float32),
        keys=rng.standard_normal((B, N, D_MODEL), dtype=np.float32),
        values=rng.standard_normal((B, N, D_MODEL), dtype=np.float32),
        boxes=rng.random((B, N, 4), dtype=np.float32),
        Wq=rng.standard_normal((H * D_K, D_MODEL), dtype=np.float32) * 0.02,
        bq=np.zeros((H * D_K,), np.float32),
        Wk=rng.standard_normal((H * D_K, D_MODEL), dtype=np.float32) * 0.02,
        bk=np.zeros((H * D_K,), np.float32),
        Wv=rng.standard_normal((H * D_V, D_MODEL), dtype=np.float32) * 0.02,
        bv=np.zeros((H * D_V,), np.float32),
        Wo=rng.standard_normal((D_MODEL, H * D_V), dtype=np.float32) * 0.02,
        bo=np.zeros((D_MODEL,), np.float32),
        Wg=rng.standard_normal((H, D_G), dtype=np.float32) * 0.02,
        bg=np.zeros((H,), np.float32),
    )
    print("demo output shape:", demo.shape, demo.dtype)


# revision 2
# speedup vs baseline: 1.0736x; 1.0736x over previous
"""Distributed kernel for nn_AugmentedGeometryScaledDotProductAttention.

Strategy: pure data-parallel over batch. B=8 batch elements map 1:1 onto the
8 trn2 NeuronCores (jax axon devices). Each core runs the full per-batch
computation (geometry bias + 16-head attention + output projection) on its
own batch element; results are gathered back to a full (8, 512, 1024) output.
No cross-core collectives are needed: fc_o only needs the 16 heads of its own
batch element, which are all resident on the same core.

Self-contained: all shapes/constants hardcoded from the problem spec.
"""

import functools

import jax
import jax.numpy as jnp
import numpy as np

D_MODEL = 1024
H = 16
D_K = 64
D_V = 64
D_G = D_MODEL // H  # 64
WAVE_LEN = 1000.0
B = 8
N = 512
N_CORES = 8


def _box_relational_embedding(boxes):
    # boxes: (n, 4) for a single batch element
    x_min, y_min, x_max, y_max = jnp.split(boxes, 4, axis=-1)  # (n, 1)
    cx = (x_min + x_max) * 0.5
    cy = (y_min + y_max) * 0.5
    w = (x_max - x_min) + 1.0
    h = (y_max - y_min) + 1.0
    delta_x = jnp.log(jnp.clip(jnp.abs((cx - cx.T) / w), 1e-3, None))
    delta_y = jnp.log(jnp.clip(jnp.abs((cy - cy.T) / h), 1e-3, None))
    delta_w = jnp.log(w / w.T)
    delta_h = jnp.log(h / h.T)
    pos = jnp.stack([delta_x, delta_y, delta_w, delta_h], axis=-1)  # (n, n, 4)
    n_freq = D_G // 8
    feat_range = jnp.arange(n_freq, dtype=jnp.float32)
    dim_mat = 1.0 / (WAVE_LEN ** (feat_range / n_freq))
    mul = (100.0 * pos)[..., None] * dim_mat  # (n, n, 4, n_freq)
    n = pos.shape[0]
    mul = mul.reshape(n, n, 4 * n_freq)
    return jnp.concatenate([jnp.sin(mul), jnp.cos(mul)], axis=-1)  # (n, n, D_G)


def _per_batch(q_in, k_in, v_in, boxes, Wq, bq, Wk, bk, Wv, bv, Wo, bo, Wg, bg):
    # q_in/k_in/v_in: (N, D_MODEL); boxes: (N, 4) — one batch element.
    # Matmuls run with bf16 operands + f32 accumulation (4x faster on the PE
    # array); the geometry/log/softmax path stays f32 (large sin args and log
    # of small clipped values need f32 inputs).
    bf = jnp.bfloat16
    f32 = jnp.float32

    def dot(x, y):
        return jax.lax.dot_general(
            x.astype(bf),
            y.astype(bf),
            (((x.ndim - 1,), (0,)), ((), ())),
            preferred_element_type=f32,
        )

    emb = _box_relational_embedding(boxes)  # (N, N, D_G) f32
    g = jax.nn.relu(
        jnp.einsum(
            "nmd,hd->hnm", emb.astype(bf), Wg.astype(bf), preferred_element_type=f32
        )
        + bg[:, None, None]
    )
    q = (dot(q_in, Wq.T) + bq).reshape(N, H, D_K).transpose(1, 0, 2)  # (H, N, D_K)
    k = (dot(k_in, Wk.T) + bk).reshape(N, H, D_K).transpose(1, 0, 2)
    v = (dot(v_in, Wv.T) + bv).reshape(N, H, D_V).transpose(1, 0, 2)
    a = jnp.einsum(
        "hqd,hkd->hqk", q.astype(bf), k.astype(bf), preferred_element_type=f32
    ) / jnp.sqrt(jnp.float32(D_K))
    mn = jax.nn.softmax(jnp.log(jnp.clip(g, 1e-6, None)) + a, axis=-1)
    out = jnp.einsum(
        "hqk,hkd->qhd", mn.astype(bf), v.astype(bf), preferred_element_type=f32
    ).reshape(N, H * D_V)
    return dot(out, Wo.T) + bo  # (N, D_MODEL)


@functools.partial(
    jax.pmap,
    axis_name="cores",
    in_axes=(0, 0, 0, 0) + (None,) * 10,
    out_axes=0,
)
def _pmapped(queries, keys, values, boxes, Wq, bq, Wk, bk, Wv, bv, Wo, bo, Wg, bg):
    return _per_batch(
        queries, keys, values, boxes, Wq, bq, Wk, bk, Wv, bv, Wo, bo, Wg, bg
    )


def kernel(
    queries, keys, values, boxes, Wq, bq, Wk, bk, Wv, bv, Wo, bo, Wg, bg
) -> np.ndarray:
    """Full inputs in, full output out. Shards batch across the 8 NeuronCores."""
    out = _pmapped(
        jnp.asarray(queries, jnp.float32),
        jnp.asarray(keys, jnp.float32),
        jnp.asarray(values, jnp.float32),
        jnp.asarray(boxes, jnp.float32),
        jnp.asarray(Wq, jnp.float32),
        jnp.asarray(bq, jnp.float32),
        jnp.asarray(Wk, jnp.float32),
        jnp.asarray(bk, jnp.float32),
        jnp.asarray(Wv, jnp.float32),
        jnp.asarray(bv, jnp.float32),
        jnp.asarray(Wo, jnp.float32),
        jnp.asarray(bo, jnp.float32),
        jnp.asarray(Wg, jnp.float32),
        jnp.asarray(bg, jnp.float32),
    )
    return np.asarray(out, dtype=np.float32)  # (B, N, D_MODEL)


if __name__ == "__main__":
    rng = np.random.default_rng(0)
    demo = kernel(
        queries=rng.standard_normal((B, N, D_MODEL), dtype=np.float32),
        keys=rng.standard_normal((B, N, D_MODEL), dtype=np.float32),
        values=rng.standard_normal((B, N, D_MODEL), dtype=np.float32),
        boxes=rng.random((B, N, 4), dtype=np.float32),
        Wq=rng.standard_normal((H * D_K, D_MODEL), dtype=np.float32) * 0.02,
        bq=np.zeros((H * D_K,), np.float32),
        Wk=rng.standard_normal((H * D_K, D_MODEL), dtype=np.float32) * 0.02,
        bk=np.zeros((H * D_K,), np.float32),
        Wv=rng.standard_normal((H * D_V, D_MODEL), dtype=np.float32) * 0.02,
        bv=np.zeros((H * D_V,), np.float32),
        Wo=rng.standard_normal((D_MODEL, H * D_V), dtype=np.float32) * 0.02,
        bo=np.zeros((D_MODEL,), np.float32),
        Wg=rng.standard_normal((H, D_G), dtype=np.float32) * 0.02,
        bg=np.zeros((H,), np.float32),
    )
    print("demo output shape:", demo.shape, demo.dtype)


# revision 3
# speedup vs baseline: 1.0996x; 1.0242x over previous
"""Distributed kernel for nn_AugmentedGeometryScaledDotProductAttention.

Strategy: pure data-parallel over batch. B=8 batch elements map 1:1 onto the
8 trn2 NeuronCores (jax axon devices). Each core runs the full per-batch
computation (geometry bias + 16-head attention + output projection) on its
own batch element; results are gathered back to a full (8, 512, 1024) output.
No cross-core collectives are needed: fc_o only needs the 16 heads of its own
batch element, which are all resident on the same core.

Self-contained: all shapes/constants hardcoded from the problem spec.
"""

import functools

import jax
import jax.numpy as jnp
import numpy as np

D_MODEL = 1024
H = 16
D_K = 64
D_V = 64
D_G = D_MODEL // H  # 64
WAVE_LEN = 1000.0
B = 8
N = 512
N_CORES = 8


def _box_relational_embedding(boxes):
    # boxes: (n, 4) for a single batch element
    x_min, y_min, x_max, y_max = jnp.split(boxes, 4, axis=-1)  # (n, 1)
    cx = (x_min + x_max) * 0.5
    cy = (y_min + y_max) * 0.5
    w = (x_max - x_min) + 1.0
    h = (y_max - y_min) + 1.0
    delta_x = jnp.log(jnp.clip(jnp.abs((cx - cx.T) / w), 1e-3, None))
    delta_y = jnp.log(jnp.clip(jnp.abs((cy - cy.T) / h), 1e-3, None))
    delta_w = jnp.log(w / w.T)
    delta_h = jnp.log(h / h.T)
    pos = jnp.stack([delta_x, delta_y, delta_w, delta_h], axis=-1)  # (n, n, 4)
    n_freq = D_G // 8
    feat_range = jnp.arange(n_freq, dtype=jnp.float32)
    dim_mat = 1.0 / (WAVE_LEN ** (feat_range / n_freq))
    mul = (100.0 * pos)[..., None] * dim_mat  # (n, n, 4, n_freq)
    n = pos.shape[0]
    mul = mul.reshape(n, n, 4 * n_freq)
    return jnp.concatenate([jnp.sin(mul), jnp.cos(mul)], axis=-1)  # (n, n, D_G)


def _per_batch(q_in, k_in, v_in, boxes, Wq, bq, Wk, bk, Wv, bv, Wo, bo, Wg, bg):
    # q_in/k_in/v_in: (N, D_MODEL); boxes: (N, 4) — one batch element.
    # Matmuls run with bf16 operands + f32 accumulation (4x faster on the PE
    # array); the geometry/log/softmax path stays f32 (large sin args and log
    # of small clipped values need f32 inputs).
    bf = jnp.bfloat16
    f32 = jnp.float32

    def dot(x, y):
        return jax.lax.dot_general(
            x.astype(bf),
            y.astype(bf),
            (((x.ndim - 1,), (0,)), ((), ())),
            preferred_element_type=f32,
        )

    emb = _box_relational_embedding(boxes)  # (N, N, D_G) f32
    g = jax.nn.relu(
        jnp.einsum(
            "nmd,hd->hnm", emb.astype(bf), Wg.astype(bf), preferred_element_type=f32
        )
        + bg[:, None, None]
    )
    q = (dot(q_in, Wq.T) + bq).reshape(N, H, D_K).transpose(1, 0, 2)  # (H, N, D_K)
    k = (dot(k_in, Wk.T) + bk).reshape(N, H, D_K).transpose(1, 0, 2)
    v = (dot(v_in, Wv.T) + bv).reshape(N, H, D_V).transpose(1, 0, 2)
    a = jnp.einsum(
        "hqd,hkd->hqk", q.astype(bf), k.astype(bf), preferred_element_type=f32
    ) / jnp.sqrt(jnp.float32(D_K))
    # softmax(log(clip(g)) + a) == g'*exp(a) / sum(g'*exp(a)): skips the log
    # over (H, N, N). a is bounded (|a| ≲ 5 for unit-scale inputs), so the
    # max-free exp is safe in f32.
    gp = jnp.clip(g, 1e-6, None)
    num = gp * jnp.exp(a)
    mn = num / jnp.sum(num, axis=-1, keepdims=True)
    out = jnp.einsum(
        "hqk,hkd->qhd", mn.astype(bf), v.astype(bf), preferred_element_type=f32
    ).reshape(N, H * D_V)
    return dot(out, Wo.T) + bo  # (N, D_MODEL)


@functools.partial(
    jax.pmap,
    axis_name="cores",
    in_axes=(0, 0, 0, 0) + (None,) * 10,
    out_axes=0,
)
def _pmapped(queries, keys, values, boxes, Wq, bq, Wk, bk, Wv, bv, Wo, bo, Wg, bg):
    return _per_batch(
        queries, keys, values, boxes, Wq, bq, Wk, bk, Wv, bv, Wo, bo, Wg, bg
    )


def kernel(
    queries, keys, values, boxes, Wq, bq, Wk, bk, Wv, bv, Wo, bo, Wg, bg
) -> np.ndarray:
    """Full inputs in, full output out. Shards batch across the 8 NeuronCores."""
    out = _pmapped(
        jnp.asarray(queries, jnp.float32),
        jnp.asarray(keys, jnp.float32),
        jnp.asarray(values, jnp.float32),
        jnp.asarray(boxes, jnp.float32),
        jnp.asarray(Wq, jnp.float32),
        jnp.asarray(bq, jnp.float32),
        jnp.asarray(Wk, jnp.float32),
        jnp.asarray(bk, jnp.float32),
        jnp.asarray(Wv, jnp.float32),
        jnp.asarray(bv, jnp.float32),
        jnp.asarray(Wo, jnp.float32),
        jnp.asarray(bo, jnp.float32),
        jnp.asarray(Wg, jnp.float32),
        jnp.asarray(bg, jnp.float32),
    )
    return np.asarray(out, dtype=np.float32)  # (B, N, D_MODEL)


if __name__ == "__main__":
    rng = np.random.default_rng(0)
    demo = kernel(
        queries=rng.standard_normal((B, N, D_MODEL), dtype=np.float32),
        keys=rng.standard_normal((B, N, D_MODEL), dtype=np.float32),
        values=rng.standard_normal((B, N, D_MODEL), dtype=np.float32),
        boxes=rng.random((B, N, 4), dtype=np.float32),
        Wq=rng.standard_normal((H * D_K, D_MODEL), dtype=np.float32) * 0.02,
        bq=np.zeros((H * D_K,), np.float32),
        Wk=rng.standard_normal((H * D_K, D_MODEL), dtype=np.float32) * 0.02,
        bk=np.zeros((H * D_K,), np.float32),
        Wv=rng.standard_normal((H * D_V, D_MODEL), dtype=np.float32) * 0.02,
        bv=np.zeros((H * D_V,), np.float32),
        Wo=rng.standard_normal((D_MODEL, H * D_V), dtype=np.float32) * 0.02,
        bo=np.zeros((D_MODEL,), np.float32),
        Wg=rng.standard_normal((H, D_G), dtype=np.float32) * 0.02,
        bg=np.zeros((H,), np.float32),
    )
    print("demo output shape:", demo.shape, demo.dtype)
